# revision 1
# baseline (speedup 1.0000x reference)
"""AttendAndSpell (LAS decoder) Trainium2 Bass kernel.

Strategy: pure data-parallel over batch (B=64 -> 8 items/core, 8 cores), no
collectives.

Layouts (per core, items b = 0..7):
  - "strided batch": item b lives on SBUF partition PB[b], PB = [0, 32, 64,
    96, 16, 48, 80, 112] (compute engines cannot address partition offsets
    that are not 32-aligned, and PE column groups are 32-aligned; merging the
    two 4-item groups at a 16-partition stride gives all 8 items in one tile).
    All states / gates / softmax tensors are [128, *] tiles, elementwise ops
    run full-tile (garbage rows are zero/bounded and never read).
  - transposed fp16 stationaries [128, kt, 128] via PE transposes: item b in
    column PB[b] (transpose of the strided rows), used as matmul lhsT.
  - attention operands: hp in two layouts (r-on-partition, d-on-partition),
    built once in phase 1; per-item score/context matmuls (M=1) are packed
    4-wide across PE column groups via tile_position.

Per step: gate matmuls are activation-stationary (fp16 weights moving, SBUF
resident); output projection is deferred (s1/c stream to DRAM as fp16,
one [T*8, 1024] @ [1024, 4096] GEMM at the end). All matmul operands fp16
(fp32 PSUM accumulation); elementwise/softmax/bias/embedding math fp32.
Measured numpy end-to-end error of the fp16 scheme: ~4e-4 rel, flat in t.
"""

import math

import numpy as np

import concourse.bacc as bacc
import concourse.mybir as mybir
import concourse.tile as tile
from concourse.bass_utils import run_bass_kernel_spmd
from concourse.masks import make_identity

B, R, T, H, V = 64, 256, 128, 512, 4096
NCORES = 8
BS = B // NCORES  # 8
G = 4 * H  # 2048
KC = 2 * H  # 1024
KT_H = H // 128  # 4
KT_KC = KC // 128  # 8
RT = R // 128  # 2
F32 = mybir.dt.float32
F16 = mybir.dt.float16
AF = mybir.ActivationFunctionType
ALU = mybir.AluOpType
AX = mybir.AxisListType

PB = [0, 32, 64, 96, 16, 48, 80, 112]  # partition of item b
PERM16 = [0, 4, 1, 5, 2, 6, 3, 7]  # row 16*j holds item PERM16[j]
INV16 = [0, 2, 4, 6, 1, 3, 5, 7]  # col INV16[b] (16-stride) holds item b


def _s16(ap):
    """[128, ...] -> 16-stride partition view [8, ...] (rows 0,16,...,112)."""
    return ap.rearrange("(i s) ... -> i s ...", s=16)[:, 0]


def build_program(T_steps=T):
    nc = bacc.Bacc(None, target_bir_lowering=False)

    d_hT = nc.dram_tensor("hT", [H, BS * R], F16, kind="ExternalInput")
    d_W0T = nc.dram_tensor("W0T", [KC, G], F16, kind="ExternalInput")
    d_W1T = nc.dram_tensor("W1T", [KC, G], F16, kind="ExternalInput")
    d_phiT = nc.dram_tensor("phiT", [H, H], F16, kind="ExternalInput")
    d_psiT = nc.dram_tensor("psiT", [H, H], F16, kind="ExternalInput")
    d_psib_bc = nc.dram_tensor("psib_bc", [128, H], F32, kind="ExternalInput")
    d_psibT = nc.dram_tensor("psibT", [128, KT_H], F32, kind="ExternalInput")
    d_phibT = nc.dram_tensor("phibT", [128, KT_H], F16, kind="ExternalInput")
    d_b1_bc = nc.dram_tensor("b1_bc", [128, G], F32, kind="ExternalInput")
    # embb rows are in PERM16 item order (row j -> partition 16*j)
    d_embb = nc.dram_tensor("embb", [T_steps, BS, G], F32, kind="ExternalInput")
    d_owT = nc.dram_tensor("owT", [KC, V], F16, kind="ExternalInput")
    d_smat = nc.dram_tensor("smat", [128, 128], F16, kind="ExternalInput")
    d_ob_bc = nc.dram_tensor("ob_bc", [128, V], F32, kind="ExternalInput")
    # out[t, j, :] is item PERM16[j]; host unscrambles + transposes
    d_out = nc.dram_tensor("out", [T_steps, BS, V], F32, kind="ExternalOutput")
    d_histT = nc.dram_tensor("histT", [T_steps, 128, KT_KC, BS], F16)

    with tile.TileContext(nc) as tc:
        with (
            tc.tile_pool(name="persist", bufs=1) as persist,
            tc.tile_pool(name="work", bufs=2) as work,
            tc.tile_pool(name="workbig", bufs=1) as workbig,
        ):
            ident = persist.tile([128, 128], F32)
            make_identity(nc, ident)

            # strided-batch fp32 state tiles
            s0 = persist.tile([128, H], F32)
            s1 = persist.tile([128, H], F32)
            cs0 = persist.tile([128, H], F32)
            cs1 = persist.tile([128, H], F32)
            cstr0 = persist.tile([128, H], F32)  # context, items 0-3 at rows 32j
            cstr1 = persist.tile([128, H], F32)  # context, items 4-7 at rows 32j
            # transposed fp16 stationaries [128, kt, col]
            s0T = persist.tile([128, KT_H, 128], F16)
            s1T = persist.tile([128, KT_H, 128], F16)
            cT = persist.tile([128, KT_H, 128], F16)
            spT = persist.tile([128, KT_H, BS], F16)  # col j = item PERM16[j]
            alT0 = persist.tile([128, RT, 128], F16)
            alT1 = persist.tile([128, RT, 128], F16)
            for st in (s0, s1, cs0, cs1, cstr0, cstr1):
                nc.vector.memset(st, 0.0)
            for st in (s0T, s1T, cT):
                nc.vector.memset(st, 0.0)

            sb_b1 = persist.tile([128, G], F32)
            nc.sync.dma_start(sb_b1, d_b1_bc[:])
            sb_smat = persist.tile([128, 128], F16)
            nc.sync.dma_start(sb_smat, d_smat[:])
            sb_psibT = persist.tile([128, KT_H], F32)
            nc.sync.dma_start(sb_psibT, d_psibT[:])
            sb_phibT = persist.tile([128, KT_H], F16)
            nc.sync.dma_start(sb_phibT, d_phibT[:])
            ring = [
                persist.tile([128, G], F32, name=f"ring{i}", tag=f"ring{i}")
                for i in range(3)
            ]
            for rg in ring:
                nc.vector.memset(rg, 0.0)

            with tc.tile_pool(name="wts", bufs=1) as wts:
                sb_W0T = wts.tile([128, KT_KC, G], F16)
                nc.sync.dma_start(sb_W0T, d_W0T.rearrange("(kt p) g -> p kt g", p=128))
                sb_W1T = wts.tile([128, KT_KC, G], F16)
                nc.sync.dma_start(sb_W1T, d_W1T.rearrange("(kt p) g -> p kt g", p=128))
                sb_phiT = wts.tile([128, KT_H, H], F16)
                nc.sync.dma_start(sb_phiT, d_phiT.rearrange("(kt p) f -> p kt f", p=128))
                sb_hp = wts.tile([128, RT * BS, H], F16)  # [p, rt*BS+b, d]
                sb_hpT = wts.tile([128, KT_H * BS, R], F16)  # [p, dt*BS+b, r]
                sb_eb0 = wts.tile([128, R], F32)  # e_base items 0-3 at rows 32j
                sb_eb1 = wts.tile([128, R], F32)

                # ---------------- Phase 1 ----------------
                with (
                    tc.tile_pool(name="ph1", bufs=1) as ph1,
                    tc.tile_pool(name="pp1", bufs=2, space="PSUM") as pp1,
                ):
                    NBR = BS * R  # 2048
                    sb_hT = ph1.tile([128, KT_H, NBR], F16)
                    nc.sync.dma_start(sb_hT, d_hT.rearrange("(kt p) n -> p kt n", p=128))
                    sb_psiT = ph1.tile([128, KT_H, H], F16)
                    nc.sync.dma_start(
                        sb_psiT, d_psiT.rearrange("(kt p) f -> p kt f", p=128)
                    )
                    sb_psib = ph1.tile([128, H], F32)
                    nc.sync.dma_start(sb_psib, d_psib_bc[:])

                    # hp (r-on-partition): act-stationary GEMM
                    for m in range(NBR // 128):  # 16
                        ps = pp1.tile([128, H], F32, tag="pp1", name="ps1")
                        for kt in range(KT_H):
                            nc.tensor.matmul(
                                ps,
                                lhsT=sb_hT[:, kt, m * 128 : (m + 1) * 128],
                                rhs=sb_psiT[:, kt, :],
                                start=(kt == 0),
                                stop=(kt == KT_H - 1),
                            )
                        b_, rt_ = divmod(m, RT)
                        nc.vector.tensor_add(sb_hp[:, rt_ * BS + b_, :], ps, sb_psib)
                    # hpT (d-on-partition): weight-stationary GEMM
                    for mt in range(KT_H):
                        for nch in range(NBR // 512):  # 4
                            ps = pp1.tile([128, H], F32, tag="pp1", name="ps2")
                            for kt in range(KT_H):
                                nc.tensor.matmul(
                                    ps,
                                    lhsT=sb_psiT[:, kt, mt * 128 : (mt + 1) * 128],
                                    rhs=sb_hT[:, kt, nch * 512 : (nch + 1) * 512],
                                    start=(kt == 0),
                                    stop=(kt == KT_H - 1),
                                )
                            for j in range(512 // R):  # 2 items per chunk
                                b_ = nch * 2 + j
                                nc.vector.tensor_scalar_add(
                                    sb_hpT[:, mt * BS + b_, :],
                                    ps[:, j * R : (j + 1) * R],
                                    sb_psibT[:, mt : mt + 1],
                                )
                    # e_base: item b -> psum tile b//4, row 32*(b%4)
                    peb0 = pp1.tile([128, H], F32, tag="pp1", name="peb0")
                    peb1 = pp1.tile([128, H], F32, tag="pp1", name="peb1")
                    nc.vector.memset(peb0, 0.0)
                    nc.vector.memset(peb1, 0.0)
                    for b_ in range(BS):
                        ps = peb0 if b_ < 4 else peb1
                        j = (b_ % 4) * 32
                        for dt in range(KT_H):
                            nc.tensor.matmul(
                                ps[j : j + 1, :R],
                                lhsT=sb_phibT[:, dt : dt + 1],
                                rhs=sb_hpT[:, dt * BS + b_, :],
                                start=(dt == 0),
                                stop=(dt == KT_H - 1),
                                tile_position=(0, j),
                            )
                    nc.vector.tensor_copy(sb_eb0, peb0[:, :R])
                    nc.vector.tensor_copy(sb_eb1, peb1[:, :R])

                _p2cms = [
                    tc.tile_pool(name="ptrans", bufs=2, space="PSUM"),
                    tc.tile_pool(name="pgate", bufs=2, space="PSUM"),
                    tc.tile_pool(name="pgatep", bufs=2, space="PSUM"),
                    tc.tile_pool(name="pattn", bufs=2, space="PSUM"),
                ]
                ptrans, pgate, pgatep, pattn = [cm.__enter__() for cm in _p2cms]

                for tpre in range(min(2, T_steps)):
                    nc.sync.dma_start(_s16(ring[tpre]), d_embb[tpre])

                def softmax_half(ef, rc):
                    """softmax over free dim of one [128, R] group (rows 32j)."""
                    mx = work.tile([128, 1], F32, tag="mx")
                    nc.vector.tensor_reduce(mx, ef, AX.X, ALU.max)
                    nc.vector.tensor_scalar_mul(mx, mx, -1.0)
                    al = work.tile([128, R], F32, tag="al")
                    nc.scalar.activation(al, ef, AF.Exp, bias=mx)
                    sm = work.tile([128, 1], F32, tag="sm")
                    nc.vector.tensor_reduce(sm, al, AX.X, ALU.add)
                    nc.vector.reciprocal(rc, sm)
                    return al

                def attention(t):
                    """score -> softmax -> context -> cT; t == -1 uses e_base."""
                    if t >= 0:
                        pe0 = pattn.tile([128, H], F32, tag="pa", name="pe0")
                        pe1 = pattn.tile([128, H], F32, tag="pa", name="pe1")
                        nc.vector.memset(pe0[:, :R], 0.0)
                        nc.vector.memset(pe1[:, :R], 0.0)
                        for b_ in range(BS):
                            ps = pe0 if b_ < 4 else pe1
                            j = (b_ % 4) * 32
                            for dt in range(KT_H):
                                nc.tensor.matmul(
                                    ps[j : j + 1, :R],
                                    lhsT=spT[:, dt, INV16[b_] : INV16[b_] + 1],
                                    rhs=sb_hpT[:, dt * BS + b_, :],
                                    start=(dt == 0),
                                    stop=(dt == KT_H - 1),
                                    tile_position=(0, j),
                                )
                        ef0 = work.tile([128, R], F32, tag="ef0")
                        ef1 = work.tile([128, R], F32, tag="ef1")
                        nc.vector.tensor_add(ef0, pe0[:, :R], sb_eb0)
                        nc.vector.tensor_add(ef1, pe1[:, :R], sb_eb1)
                    else:
                        ef0, ef1 = sb_eb0, sb_eb1
                    rc0 = work.tile([128, 1], F32, tag="rc0")
                    rc1 = work.tile([128, 1], F32, tag="rc1")
                    al0 = softmax_half(ef0, rc0)
                    al1 = softmax_half(ef1, rc1)
                    # alT: transpose unnormalized alpha -> item at col 32j
                    for alx, alTx in ((al0, alT0), (al1, alT1)):
                        pta = ptrans.tile([128, KT_H, 128], F32, tag="tr", name="pta")
                        for rt_ in range(RT):
                            nc.tensor.transpose(
                                pta[:, rt_, :],
                                alx[:, rt_ * 128 : (rt_ + 1) * 128],
                                ident,
                            )
                        nc.vector.tensor_copy(alTx, pta[:, :RT, :])
                    # context (unnormalized): item b -> tile b//4, row 32*(b%4)
                    pc0 = pattn.tile([128, H], F32, tag="pa", name="pc0")
                    pc1 = pattn.tile([128, H], F32, tag="pa", name="pc1")
                    nc.vector.memset(pc0, 0.0)
                    nc.vector.memset(pc1, 0.0)
                    for b_ in range(BS):
                        ps = pc0 if b_ < 4 else pc1
                        j = (b_ % 4) * 32
                        alTx = alT0 if b_ < 4 else alT1
                        for rt_ in range(RT):
                            nc.tensor.matmul(
                                ps[j : j + 1, :],
                                lhsT=alTx[:, rt_, j : j + 1],
                                rhs=sb_hp[:, rt_ * BS + b_, :],
                                start=(rt_ == 0),
                                stop=(rt_ == RT - 1),
                                tile_position=(0, j),
                            )
                    # normalize into strided context tiles
                    nc.vector.tensor_scalar_mul(cstr0, pc0, rc0)
                    nc.vector.tensor_scalar_mul(cstr1, pc1, rc1)
                    # cT: item b at col PB[b] (32j from cstr0, 16+32j from cstr1)
                    for csx, sh in ((cstr0, 0), (cstr1, 16)):
                        ptc = ptrans.tile([128, KT_H, 128], F32, tag="tr", name="ptc")
                        for chk in range(KT_H):
                            nc.tensor.transpose(
                                ptc[:, chk, :],
                                csx[:, chk * 128 : (chk + 1) * 128],
                                ident,
                            )
                        src = ptc.rearrange("p k (i s) -> p k i s", s=32)[:, :, :, 0:1]
                        dst = cT.rearrange("p k (i s) -> p k i s", s=32)[
                            :, :, :, sh : sh + 1
                        ]
                        nc.vector.tensor_copy(dst, src)

                def lstm_cell(gact, cs, s_out):
                    i_s = gact[:, 0:H]
                    f_s = gact[:, H : 2 * H]
                    o_s = gact[:, 2 * H : 3 * H]
                    g_s = gact[:, 3 * H : 4 * H]
                    ig = work.tile([128, H], F32, tag="ig")
                    nc.vector.tensor_mul(ig, i_s, g_s)
                    nc.vector.tensor_mul(cs, f_s, cs)
                    nc.vector.tensor_add(cs, cs, ig)
                    tch = work.tile([128, H], F32, tag="tch")
                    nc.scalar.activation(tch, cs, AF.Tanh)
                    nc.vector.tensor_mul(s_out, o_s, tch)

                def transpose_state(src, dstT):
                    pt = ptrans.tile([128, KT_H, 128], F32, tag="tr", name="ptc2")
                    for chk in range(KT_H):
                        nc.tensor.transpose(
                            pt[:, chk, :], src[:, chk * 128 : (chk + 1) * 128], ident
                        )
                    nc.vector.tensor_copy(dstT, pt)

                def _compact(lhsT_x, kt):
                    # [128, kt, 128] -> [128, 32] (every 4th col; items land at
                    # compact positions 4m so the M=32 matmul writes the whole
                    # 32-row group -- no uninitialized psum rows)
                    return lhsT_x[:, kt, :].rearrange("p (i s) -> p i s", s=4)[
                        :, :, 0
                    ]

                def gates(lhsT_first, lhsT_second, W, bias_tile, gact):
                    """gact = act(W[:512rows].lhsT_first + W[512:].lhsT_second + b).

                    The 8 k-tiles are packed 4-wide across PE column groups
                    (2 accumulation rounds); the four 32-strided partial rows
                    are then summed by one matmul against the host-built
                    selection matrix S (which also lands items on their PB
                    partitions).
                    """
                    for ch in range(4):
                        ps = pgate.tile([128, 512], F32, tag="pg", name="pg")
                        for kt in range(KT_H):
                            nc.tensor.matmul(
                                ps,
                                lhsT=lhsT_second[:, kt, :],
                                rhs=W[:, KT_H + kt, ch * 512 : (ch + 1) * 512],
                                start=(kt == 0),
                                stop=False,
                            )
                        for kt in range(KT_H):
                            nc.tensor.matmul(
                                ps,
                                lhsT=lhsT_first[:, kt, :],
                                rhs=W[:, kt, ch * 512 : (ch + 1) * 512],
                                start=False,
                                stop=(kt == KT_H - 1),
                            )
                        gsl = gact[:, ch * 512 : (ch + 1) * 512]
                        nc.vector.tensor_add(
                            gsl, ps, bias_tile[:, ch * 512 : (ch + 1) * 512]
                        )
                        nc.scalar.activation(
                            gsl, gsl, AF.Sigmoid if ch < 3 else AF.Tanh
                        )

                # ---------------- Phase 2 ----------------
                attention(-1)  # c_init

                for t in range(T_steps):
                    g0a = workbig.tile([128, G], F32, tag="g0a")
                    gates(cT, s0T, sb_W0T, ring[t % 3], g0a)
                    lstm_cell(g0a, cs0, s0)
                    transpose_state(s0, s0T)

                    g1a = workbig.tile([128, G], F32, tag="g1a")
                    gates(s0T, s1T, sb_W1T, sb_b1, g1a)
                    lstm_cell(g1a, cs1, s1)
                    transpose_state(s1, s1T)

                    # spT = (phi_w/sqrt(H)) @ s1, cols in PERM16 item order
                    ptsp = ptrans.tile([128, KT_H, 128], F32, tag="tr", name="ptsp")
                    s1T16 = s1T.rearrange("p k (i s) -> p k i s", s=16)[:, :, :, 0]
                    for mt in range(KT_H):
                        for kt in range(KT_H):
                            nc.tensor.matmul(
                                ptsp[:, mt, :BS],
                                lhsT=sb_phiT[:, kt, mt * 128 : (mt + 1) * 128],
                                rhs=s1T16[:, kt, :],
                                start=(kt == 0),
                                stop=(kt == KT_H - 1),
                            )
                    nc.vector.tensor_copy(spT, ptsp[:, :, :BS])

                    attention(t)

                    nc.sync.dma_start(
                        d_histT[t, :, 0:KT_H, :],
                        s1T.rearrange("p k (i s) -> p k i s", s=16)[:, :, :, 0],
                    )
                    nc.sync.dma_start(
                        d_histT[t, :, KT_H : 2 * KT_H, :],
                        cT.rearrange("p k (i s) -> p k i s", s=16)[:, :, :, 0],
                    )
                    if t + 2 < T_steps:
                        nc.sync.dma_start(_s16(ring[(t + 2) % 3]), d_embb[t + 2])

                for cm in reversed(_p2cms):
                    cm.__exit__(None, None, None)

            # ---------------- Phase 3: output projection ----------------
            with (
                tc.tile_pool(name="ph3", bufs=1) as ph3,
                tc.tile_pool(name="ph3w", bufs=2) as ph3w,
                tc.tile_pool(name="pp3", bufs=4, space="PSUM") as pp3,
            ):
                NTB = T_steps * BS
                sb_hist = ph3.tile([128, KT_KC, NTB], F16)
                hist_v = d_histT.rearrange("t p k b -> p k t b")
                for kt in range(KT_KC):
                    nc.sync.dma_start(
                        sb_hist[:, kt, :].rearrange("p (t b) -> p t b", b=BS),
                        hist_v[:, kt, :, :],
                    )
                sb_ob = ph3.tile([128, V], F32)
                nc.sync.dma_start(sb_ob, d_ob_bc[:])
                out_tb = d_out.rearrange("t b v -> (t b) v")
                owT_v = d_owT.rearrange("(k p) v -> p k v", p=128)
                for nch in range(V // 512):  # 8
                    rhs = ph3w.tile([128, KT_KC, 512], F16, tag="owr", name="owr")
                    for kt in range(KT_KC):
                        nc.sync.dma_start(
                            rhs[:, kt, :], owT_v[:, kt, nch * 512 : (nch + 1) * 512]
                        )
                    for m in range(NTB // 128):
                        ps = pp3.tile([128, 512], F32, tag="po", name="po")
                        for kt in range(KT_KC):
                            nc.tensor.matmul(
                                ps,
                                lhsT=sb_hist[:, kt, m * 128 : (m + 1) * 128],
                                rhs=rhs[:, kt, :],
                                start=(kt == 0),
                                stop=(kt == KT_KC - 1),
                            )
                        ost = ph3w.tile([128, 512], F32, tag="ost", name="ost")
                        nc.vector.tensor_add(
                            ost, ps, sb_ob[:, nch * 512 : (nch + 1) * 512]
                        )
                        nc.sync.dma_start(
                            out_tb[m * 128 : (m + 1) * 128, nch * 512 : (nch + 1) * 512],
                            ost,
                        )
    nc.compile()
    return nc


def host_prep(inputs, T_steps=T):
    f = lambda k: np.asarray(inputs[k], np.float32)
    h = f("h")
    y = np.asarray(inputs["y"])
    w_ih0, w_hh0 = f("w_ih0"), f("w_hh0")
    b_ih0, b_hh0 = f("b_ih0"), f("b_hh0")
    w_ih1, w_hh1 = f("w_ih1"), f("w_hh1")
    b_ih1, b_hh1 = f("b_ih1"), f("b_hh1")
    phi_w, phi_b = f("phi_w"), f("phi_b")
    psi_w, psi_b = f("psi_w"), f("psi_b")
    out_w, out_b = f("out_w"), f("out_b")

    scale = 1.0 / math.sqrt(H)
    # gate reorder i,f,g,o -> i,f,o,g
    perm = np.concatenate(
        [np.arange(H), H + np.arange(H), 3 * H + np.arange(H), 2 * H + np.arange(H)]
    )
    W0T = np.concatenate([w_ih0[:, V:], w_hh0], axis=1)[perm].T  # [1024, 2048]
    W1T = np.concatenate([w_ih1, w_hh1], axis=1)[perm].T
    phiT = (phi_w * scale).T
    psiT = psi_w.T
    psibT = np.ascontiguousarray(psi_b.reshape(KT_H, 128).T)
    psib_bc = np.ascontiguousarray(np.tile(psi_b[None, :], (128, 1)))
    phibT = np.ascontiguousarray(
        ((phi_b * scale).reshape(KT_H, 128).T).astype(np.float16)
    )
    b1_bc = np.ascontiguousarray(np.tile((b_ih1 + b_hh1)[perm][None, :], (128, 1)))
    b0 = (b_ih0 + b_hh0)[perm]
    embW = w_ih0[:, :V][perm]  # [2048, 4096]
    embb_all = embW.T[y[:, :T_steps]] + b0  # [B, T, 2048]
    embb_all = np.ascontiguousarray(embb_all.transpose(1, 0, 2))  # [T, B, 2048]
    owT = out_w.T
    ob_bc = np.ascontiguousarray(np.tile(out_b[None, :], (128, 1)))
    smat = np.zeros((128, 128), np.float16)
    for j in range(4):
        for m in range(BS):
            smat[32 * j + 4 * m, 16 * m] = 1.0

    c16 = lambda x: np.ascontiguousarray(x.astype(np.float16))
    shared = dict(
        W0T=c16(W0T), W1T=c16(W1T), phiT=c16(phiT), psiT=c16(psiT),
        psib_bc=psib_bc, psibT=psibT, phibT=phibT,
        b1_bc=b1_bc, owT=c16(owT), ob_bc=ob_bc, smat=smat,
    )
    in_maps = []
    for ci in range(NCORES):
        sl = slice(ci * BS, (ci + 1) * BS)
        m = dict(shared)
        m["hT"] = c16(h[sl].reshape(BS * R, H).T)
        m["embb"] = np.ascontiguousarray(embb_all[:, sl, :][:, PERM16, :])
        in_maps.append(m)
    return in_maps


def gather_output(per_core_outs):
    """per-core device outs [T, 8(PERM16 order), V] -> [B, T, V]."""
    shards = []
    for o in per_core_outs:
        shards.append(np.ascontiguousarray(o[:, INV16, :].transpose(1, 0, 2)))
    return np.concatenate(shards, axis=0)


def kernel(**inputs):
    nc = build_program(T)
    in_maps = host_prep(inputs, T)
    res = run_bass_kernel_spmd(nc, in_maps, list(range(NCORES)))
    out = gather_output([res.results[ci]["out"] for ci in range(NCORES)])
    return np.ascontiguousarray(out.astype(np.float32))



# revision 7
# speedup vs baseline: 3.3036x; 3.3036x over previous
"""AttendAndSpell (LAS decoder) Trainium2 Bass kernel, v2.

Data-parallel over batch (B=64 -> 8 items/core on 8 cores), no collectives.

v2 design (vs v1 baseline):
  - Gate matmuls in fp8e4m3 with DoubleRow perf mode (2 fp8 MACs/cell/cycle):
    activation-stationary, stationary = transposed states/context packed as
    [128, 2, 16] fp8 plane pairs (K=256 per instruction), moving = weights
    [128, 2, 512] fp8.  Host scales weights by S=512 to dodge fp8 subnormals;
    the activation applies 1/S.  Numpy-validated end-to-end rel err ~5e-3.
  - tanh-form gates: sigmoid(z) = (tanh(z/2)+1)/2 with the 1/2 folded into
    host weights; states stored as 2*s (compensated in phi/out_w/W host-side).
    Only {Tanh, Exp} activation functions are used -> a single act table
    (exp_and_others) for the whole program, zero per-step table loads.
  - LSTM cell via fused scalar_tensor_tensor ops (4 DVE + 1 ACT per cell).
  - Gate bias / embedding enter the PSUM accumulation group as tiny fp16
    matmuls (selection-matrix trick), eliminating per-step DVE bias adds.
  - Items live on contiguous partitions 0..7 (16-padded); per-step state
    transposes are [16,128] fp16 PE transposes (cheap) + small fp8 copies.
  - Attention (score/softmax/context) kept fp16/fp32 as v1: per-item M=1
    matmuls packed 4-wide across PE column groups via tile_position.
  - embb ring and final output in fp16; output projection deferred to one
    [T*8, 1024] @ [1024, 4096] GEMM at the end (as v1).
"""

import math

import numpy as np
import ml_dtypes

import concourse.bacc as bacc
import concourse.mybir as mybir
import concourse.tile as tile
from concourse.bass_utils import run_bass_kernel_spmd
from concourse.masks import make_identity

B, R, T, H, V = 64, 256, 128, 512, 4096
NCORES = 8
BS = B // NCORES  # 8
G = 4 * H  # 2048
KC = 2 * H  # 1024
KT_H = H // 128  # 4
KT_KC = KC // 128  # 8
RT = R // 128  # 2
S = 512.0  # fp8 weight scale
F32 = mybir.dt.float32
F16 = mybir.dt.float16
E4 = mybir.dt.float8e4
AF = mybir.ActivationFunctionType
ALU = mybir.AluOpType
AX = mybir.AxisListType
DR = mybir.MatmulPerfMode.DoubleRow


def build_program(T_steps=T):
    nc = bacc.Bacc(None, target_bir_lowering=False)

    d_hT = nc.dram_tensor("hT", [H, BS * R], F16, kind="ExternalInput")
    d_W0 = nc.dram_tensor("W0", [128, 4, 2, G], E4, kind="ExternalInput")
    d_W1 = nc.dram_tensor("W1", [128, 4, 2, G], E4, kind="ExternalInput")
    d_phiT = nc.dram_tensor("phiT", [H, H], F16, kind="ExternalInput")
    d_psiT = nc.dram_tensor("psiT", [H, H], F16, kind="ExternalInput")
    d_psib_bc = nc.dram_tensor("psib_bc", [128, H], F32, kind="ExternalInput")
    d_psibT = nc.dram_tensor("psibT", [128, KT_H], F32, kind="ExternalInput")
    d_phibT = nc.dram_tensor("phibT", [128, KT_H], F16, kind="ExternalInput")
    d_sel8 = nc.dram_tensor("sel8", [BS, 16], F16, kind="ExternalInput")
    d_one8 = nc.dram_tensor("one8", [1, 16], F16, kind="ExternalInput")
    d_b1row = nc.dram_tensor("b1row", [1, G], F16, kind="ExternalInput")
    d_embT = nc.dram_tensor("embT", [T_steps, BS, G], F16, kind="ExternalInput")
    d_owT = nc.dram_tensor("owT", [KC, V], F16, kind="ExternalInput")
    d_ob_bc = nc.dram_tensor("ob_bc", [128, V], F32, kind="ExternalInput")
    d_out = nc.dram_tensor("out", [T_steps * BS, V], F16, kind="ExternalOutput")
    d_histT = nc.dram_tensor("histT", [T_steps, 128, KT_KC, BS], F16)

    with tile.TileContext(nc) as tc:
        with (
            tc.tile_pool(name="persist", bufs=1) as persist,
            tc.tile_pool(name="work", bufs=2) as work,
        ):
            identF = persist.tile([128, 128], F16)
            make_identity(nc, identF)

            # persistent state (items on partitions 0..15, 8 real)
            CS0 = persist.tile([16, H], F32)
            CS1 = persist.tile([16, H], F32)
            # transposed stationaries
            s0T8 = persist.tile([128, KT_H, 16], E4)
            s1T8 = persist.tile([128, KT_H, 16], E4)
            cT8 = persist.tile([128, KT_H, 16], E4)
            s1T16 = persist.tile([128, KT_H, BS], F16)
            cT16 = persist.tile([128, KT_H, BS], F16)
            spT = persist.tile([128, KT_H, BS], F16)
            alT0 = persist.tile([128, RT, 128], F16)
            alT1 = persist.tile([128, RT, 128], F16)
            for st in (CS0, CS1):
                nc.vector.memset(st, 0.0)
            for st in (s0T8, s1T8, cT8):
                nc.vector.memset(st, 0.0)

            sb_sel8 = persist.tile([BS, 16], F16)
            nc.sync.dma_start(sb_sel8, d_sel8[:])
            sb_one8 = persist.tile([1, 16], F16)
            nc.sync.dma_start(sb_one8, d_one8[:])
            sb_b1row = persist.tile([1, G], F16)
            nc.sync.dma_start(sb_b1row, d_b1row[:])
            sb_psibT = persist.tile([128, KT_H], F32)
            nc.sync.dma_start(sb_psibT, d_psibT[:])
            sb_phibT = persist.tile([128, KT_H], F16)
            nc.sync.dma_start(sb_phibT, d_phibT[:])
            ring = [
                persist.tile([BS, G], F16, name=f"ring{i}", tag=f"ring{i}")
                for i in range(3)
            ]

            with tc.tile_pool(name="wts", bufs=1) as wts:
                sb_W0 = wts.tile([128, 4, 2, G], E4)
                nc.sync.dma_start(sb_W0, d_W0[:])
                sb_W1 = wts.tile([128, 4, 2, G], E4)
                nc.sync.dma_start(sb_W1, d_W1[:])
                sb_phiT = wts.tile([128, KT_H, H], F16)
                nc.sync.dma_start(sb_phiT, d_phiT.rearrange("(kt p) f -> p kt f", p=128))
                sb_hp = wts.tile([128, RT * BS, H], F16)  # [p, rt*BS+b, d]
                sb_hpT = wts.tile([128, KT_H * BS, R], F16)  # [p, dt*BS+b, r]
                sb_eb0 = wts.tile([128, R], F32)  # e_base items 0-3 at rows 32j
                sb_eb1 = wts.tile([128, R], F32)

                # ---------------- Phase 1 ----------------
                with (
                    tc.tile_pool(name="ph1", bufs=1) as ph1,
                    tc.tile_pool(name="pp1", bufs=2, space="PSUM") as pp1,
                ):
                    NBR = BS * R  # 2048
                    sb_hT = ph1.tile([128, KT_H, NBR], F16)
                    nc.sync.dma_start(sb_hT, d_hT.rearrange("(kt p) n -> p kt n", p=128))
                    sb_psiT = ph1.tile([128, KT_H, H], F16)
                    nc.sync.dma_start(
                        sb_psiT, d_psiT.rearrange("(kt p) f -> p kt f", p=128)
                    )
                    sb_psib = ph1.tile([128, H], F32)
                    nc.sync.dma_start(sb_psib, d_psib_bc[:])

                    # hp (r-on-partition): act-stationary GEMM
                    for m in range(NBR // 128):  # 16
                        ps = pp1.tile([128, H], F32, tag="pp1", name="ps1")
                        for kt in range(KT_H):
                            nc.tensor.matmul(
                                ps,
                                lhsT=sb_hT[:, kt, m * 128 : (m + 1) * 128],
                                rhs=sb_psiT[:, kt, :],
                                start=(kt == 0),
                                stop=(kt == KT_H - 1),
                            )
                        b_, rt_ = divmod(m, RT)
                        nc.vector.tensor_add(sb_hp[:, rt_ * BS + b_, :], ps, sb_psib)
                    # hpT (d-on-partition): weight-stationary GEMM
                    for mt in range(KT_H):
                        for nch in range(NBR // 512):  # 4
                            ps = pp1.tile([128, H], F32, tag="pp1", name="ps2")
                            for kt in range(KT_H):
                                nc.tensor.matmul(
                                    ps,
                                    lhsT=sb_psiT[:, kt, mt * 128 : (mt + 1) * 128],
                                    rhs=sb_hT[:, kt, nch * 512 : (nch + 1) * 512],
                                    start=(kt == 0),
                                    stop=(kt == KT_H - 1),
                                )
                            for j in range(512 // R):  # 2 items per chunk
                                b_ = nch * 2 + j
                                nc.vector.tensor_scalar_add(
                                    sb_hpT[:, mt * BS + b_, :],
                                    ps[:, j * R : (j + 1) * R],
                                    sb_psibT[:, mt : mt + 1],
                                )
                    # e_base: item b -> psum tile b//4, row 32*(b%4)
                    peb0 = pp1.tile([128, H], F32, tag="pp1", name="peb0")
                    peb1 = pp1.tile([128, H], F32, tag="pp1", name="peb1")
                    nc.vector.memset(peb0, 0.0)
                    nc.vector.memset(peb1, 0.0)
                    for b_ in range(BS):
                        ps = peb0 if b_ < 4 else peb1
                        j = (b_ % 4) * 32
                        for dt in range(KT_H):
                            nc.tensor.matmul(
                                ps[j : j + 1, :R],
                                lhsT=sb_phibT[:, dt : dt + 1],
                                rhs=sb_hpT[:, dt * BS + b_, :],
                                start=(dt == 0),
                                stop=(dt == KT_H - 1),
                                tile_position=(0, j),
                            )
                    nc.vector.tensor_copy(sb_eb0, peb0[:, :R])
                    nc.vector.tensor_copy(sb_eb1, peb1[:, :R])

                _p2cms = [
                    tc.tile_pool(name="pgate", bufs=2, space="PSUM"),
                    tc.tile_pool(name="pmix", bufs=1, space="PSUM"),
                    tc.tile_pool(name="pc", bufs=2, space="PSUM"),
                    tc.tile_pool(name="psmall", bufs=1, space="PSUM"),
                ]
                pgate, pmix, pcp, psmall = [cm.__enter__() for cm in _p2cms]

                for tpre in range(min(2, T_steps)):
                    nc.sync.dma_start(ring[tpre], d_embT[tpre])

                def softmax_half(ef, rc, alT):
                    """ef [128, R] f32 -> unnormalized alpha fp16 -> alT."""
                    mx = work.tile([128, 1], F32, tag="mx")
                    nc.vector.tensor_reduce(mx, ef, AX.X, ALU.max)
                    nc.vector.tensor_scalar_mul(mx, mx, -1.0)
                    al = work.tile([128, R], F16, tag="al")
                    nc.scalar.activation(al, ef, AF.Exp, bias=mx)
                    sm = work.tile([128, 1], F32, tag="sm")
                    nc.vector.tensor_reduce(sm, al, AX.X, ALU.add)
                    nc.vector.reciprocal(rc, sm)
                    pta = psmall.tile([128, RT, 128], F16, tag="sh", name="pta")
                    for rt_ in range(RT):
                        nc.tensor.transpose(
                            pta[:, rt_, :], al[:, rt_ * 128 : (rt_ + 1) * 128], identF
                        )
                    nc.vector.tensor_copy(alT, pta)

                def attention(t):
                    """score -> softmax -> context -> cT8/cT16; t==-1 uses e_base."""
                    if t >= 0:
                        pe = pmix.tile([128, 2, R], F32, tag="mx", name="pe")
                        for b_ in range(BS):
                            g = b_ // 4
                            j = (b_ % 4) * 32
                            for dt in range(KT_H):
                                nc.tensor.matmul(
                                    pe[j : j + 1, g, :],
                                    lhsT=spT[:, dt, b_ : b_ + 1],
                                    rhs=sb_hpT[:, dt * BS + b_, :],
                                    start=(dt == 0),
                                    stop=(dt == KT_H - 1),
                                    tile_position=(0, j),
                                )
                        ef0 = work.tile([128, R], F32, tag="ef0")
                        ef1 = work.tile([128, R], F32, tag="ef1")
                        nc.vector.tensor_add(ef0, pe[:, 0, :], sb_eb0)
                        nc.vector.tensor_add(ef1, pe[:, 1, :], sb_eb1)
                    else:
                        ef0, ef1 = sb_eb0, sb_eb1
                    rc0 = work.tile([128, 1], F32, tag="rc0")
                    rc1 = work.tile([128, 1], F32, tag="rc1")
                    softmax_half(ef0, rc0, alT0)
                    softmax_half(ef1, rc1, alT1)
                    # context (unnormalized): item b -> tile b//4, row 32*(b%4)
                    pc0 = pcp.tile([128, H], F32, tag="pc", name="pc0")
                    pc1 = pcp.tile([128, H], F32, tag="pc", name="pc1")
                    for b_ in range(BS):
                        ps = pc0 if b_ < 4 else pc1
                        j = (b_ % 4) * 32
                        alTx = alT0 if b_ < 4 else alT1
                        for rt_ in range(RT):
                            nc.tensor.matmul(
                                ps[j : j + 1, :],
                                lhsT=alTx[:, rt_, j : j + 1],
                                rhs=sb_hp[:, rt_ * BS + b_, :],
                                start=(rt_ == 0),
                                stop=(rt_ == RT - 1),
                                tile_position=(0, j),
                            )
                    # normalize (strided rows; garbage rows bounded-ignored)
                    cstr0 = work.tile([128, H], F16, tag="cstr0")
                    cstr1 = work.tile([128, H], F16, tag="cstr1")
                    nc.vector.tensor_scalar_mul(cstr0, pc0, rc0)
                    nc.vector.tensor_scalar_mul(cstr1, pc1, rc1)
                    # transpose: item at col 32j -> cT8/cT16 compact cols
                    for gi, csx in ((0, cstr0), (1, cstr1)):
                        ptc = pmix.tile([128, KT_H, 128], F16, tag="mx", name="ptc")
                        for chk in range(KT_H):
                            nc.tensor.transpose(
                                ptc[:, chk, :],
                                csx[:, chk * 128 : (chk + 1) * 128],
                                identF,
                            )
                        src = ptc.rearrange("p k (i s) -> p k i s", s=32)[:, :, :, 0]
                        nc.vector.tensor_copy(cT8[:, :, gi * 4 : gi * 4 + 4], src)
                        nc.vector.tensor_copy(cT16[:, :, gi * 4 : gi * 4 + 4], src)

                def gates(W, first8, second8, bias_lhsT, bias_rhs, gact):
                    """psum group: bias MM + 4 DoubleRow MMs per 512-col chunk.

                    K order: skt 0,1 = `first` input dims, skt 2,3 = `second`;
                    `second` (older state) MMs issue before `first`.
                    """
                    for ch in range(4):
                        ps = pgate.tile([16, 512], F32, tag="pg", name="pg")
                        csl = slice(ch * 512, (ch + 1) * 512)
                        nc.tensor.matmul(
                            ps, lhsT=bias_lhsT, rhs=bias_rhs[:, csl],
                            start=True, stop=False,
                        )
                        for skt in (2, 3, 0, 1):
                            lhsT = (second8 if skt >= 2 else first8)[
                                :, 2 * (skt % 2) : 2 * (skt % 2) + 2, :
                            ]
                            nc.tensor.matmul(
                                ps,
                                lhsT=lhsT,
                                rhs=W[:, skt, :, csl],
                                start=False,
                                stop=(skt == 1),
                                perf_mode=DR,
                            )
                        nc.scalar.activation(
                            gact[:, csl], ps, AF.Tanh, scale=1.0 / S
                        )

                def cell(gact, CS, S2):
                    ti = gact[:, 0:H]
                    tf = gact[:, H : 2 * H]
                    to = gact[:, 2 * H : 3 * H]
                    tg = gact[:, 3 * H : 4 * H]
                    Bv = work.tile([16, H], F16, tag="Bv")
                    nc.vector.scalar_tensor_tensor(Bv, ti, 1.0, tg, ALU.add, ALU.mult)
                    Av = work.tile([16, H], F32, tag="Av")
                    nc.vector.scalar_tensor_tensor(Av, tf, 1.0, CS, ALU.add, ALU.mult)
                    nc.vector.scalar_tensor_tensor(CS, Av, 0.5, Bv, ALU.mult, ALU.add)
                    tch = work.tile([16, H], F16, tag="tch")
                    nc.scalar.activation(tch, CS, AF.Tanh, scale=0.5)
                    nc.vector.scalar_tensor_tensor(S2, to, 1.0, tch, ALU.add, ALU.mult)

                def transpose_state(S2, outs):
                    """S2 [16, 512] f16 -> [128, kt, 16] psum -> copies."""
                    pt = psmall.tile([128, KT_H, 16], F16, tag="sh", name="pt")
                    for k in range(KT_H):
                        nc.tensor.transpose(
                            pt[:, k, :], S2[:, k * 128 : (k + 1) * 128],
                            identF[0:16, 0:16],
                        )
                    for dst, w in outs:
                        nc.vector.tensor_copy(dst, pt[:, :, :w])

                # ---------------- Phase 2 ----------------
                attention(-1)  # c_init

                gact0 = persist.tile([16, G], F16)
                gact1 = persist.tile([16, G], F16)
                S2_0 = persist.tile([16, H], F16)
                S2_1 = persist.tile([16, H], F16)

                for t in range(T_steps):
                    gates(sb_W0, cT8, s0T8, sb_sel8, ring[t % 3], gact0)
                    cell(gact0, CS0, S2_0)
                    transpose_state(S2_0, [(s0T8, 16)])

                    gates(sb_W1, s0T8, s1T8, sb_one8, sb_b1row, gact1)
                    cell(gact1, CS1, S2_1)
                    transpose_state(S2_1, [(s1T8, 16), (s1T16, BS)])

                    # spT = (phi_w*scale*0.5) @ (2*s1)
                    ptsp = psmall.tile([128, KT_H, BS], F32, tag="sh", name="ptsp")
                    for mt in range(KT_H):
                        for kt in range(KT_H):
                            nc.tensor.matmul(
                                ptsp[:, mt, :],
                                lhsT=sb_phiT[:, kt, mt * 128 : (mt + 1) * 128],
                                rhs=s1T16[:, kt, :],
                                start=(kt == 0),
                                stop=(kt == KT_H - 1),
                            )
                    nc.vector.tensor_copy(spT, ptsp)

                    attention(t)

                    nc.sync.dma_start(d_histT[t, :, 0:KT_H, :], s1T16)
                    nc.sync.dma_start(d_histT[t, :, KT_H : 2 * KT_H, :], cT16)
                    if t + 2 < T_steps:
                        nc.sync.dma_start(ring[(t + 2) % 3], d_embT[t + 2])

                for cm in reversed(_p2cms):
                    cm.__exit__(None, None, None)

            # ---------------- Phase 3: output projection ----------------
            with (
                tc.tile_pool(name="ph3", bufs=1) as ph3,
                tc.tile_pool(name="ph3w", bufs=2) as ph3w,
                tc.tile_pool(name="pp3", bufs=4, space="PSUM") as pp3,
            ):
                NTB = T_steps * BS
                sb_hist = ph3.tile([128, KT_KC, NTB], F16)
                hist_v = d_histT.rearrange("t p k b -> p k t b")
                for kt in range(KT_KC):
                    nc.sync.dma_start(
                        sb_hist[:, kt, :].rearrange("p (t b) -> p t b", b=BS),
                        hist_v[:, kt, :, :],
                    )
                sb_ob = ph3.tile([128, V], F32)
                nc.sync.dma_start(sb_ob, d_ob_bc[:])
                owT_v = d_owT.rearrange("(k p) v -> p k v", p=128)
                for nch in range(V // 512):  # 8
                    rhs = ph3w.tile([128, KT_KC, 512], F16, tag="owr", name="owr")
                    for kt in range(KT_KC):
                        nc.sync.dma_start(
                            rhs[:, kt, :], owT_v[:, kt, nch * 512 : (nch + 1) * 512]
                        )
                    for m in range(NTB // 128):
                        ps = pp3.tile([128, 512], F32, tag="po", name="po")
                        for kt in range(KT_KC):
                            nc.tensor.matmul(
                                ps,
                                lhsT=sb_hist[:, kt, m * 128 : (m + 1) * 128],
                                rhs=rhs[:, kt, :],
                                start=(kt == 0),
                                stop=(kt == KT_KC - 1),
                            )
                        ost = ph3w.tile([128, 512], F16, tag="ost", name="ost")
                        nc.vector.tensor_add(
                            ost, ps, sb_ob[:, nch * 512 : (nch + 1) * 512]
                        )
                        nc.sync.dma_start(
                            d_out[m * 128 : (m + 1) * 128, nch * 512 : (nch + 1) * 512],
                            ost,
                        )
    nc.compile()
    return nc


def host_prep(inputs, T_steps=T):
    f = lambda k: np.asarray(inputs[k], np.float32)
    h = f("h")
    y = np.asarray(inputs["y"])
    scale = 1.0 / math.sqrt(H)
    # gate reorder i,f,g,o -> i,f,o,g; i/f/o rows x0.5 (tanh-form sigmoid)
    perm = np.concatenate(
        [np.arange(H), H + np.arange(H), 3 * H + np.arange(H), 2 * H + np.arange(H)]
    )
    gs = np.concatenate([np.full(3 * H, 0.5), np.ones(H)]).astype(np.float32)[:, None]
    w_ih0, w_hh0 = f("w_ih0")[perm], f("w_hh0")[perm]
    w_ih1, w_hh1 = f("w_ih1")[perm], f("w_hh1")[perm]
    b0 = (f("b_ih0") + f("b_hh0"))[perm]
    b1 = (f("b_ih1") + f("b_hh1"))[perm]
    # state inputs are stored as 2*s -> their weight columns x0.5
    W0 = np.concatenate([w_ih0[:, V:], w_hh0 * 0.5], axis=1) * gs  # [G, KC]
    W1 = np.concatenate([w_ih1 * 0.5, w_hh1 * 0.5], axis=1) * gs

    def pack8(Wm):  # [G, KC] -> [128, skt 4, plane 2, G] fp8 (scaled by S)
        Wt = np.ascontiguousarray(Wm.T * S)  # [KC, G]
        return np.ascontiguousarray(
            Wt.reshape(4, 2, 128, G).transpose(2, 0, 1, 3)
        ).astype(ml_dtypes.float8_e4m3)

    embW = w_ih0[:, :V] * gs
    emb_all = (embW.T[y[:, :T_steps]] + (b0 * gs[:, 0])[None, None, :]) * S
    embT = np.ascontiguousarray(emb_all.transpose(1, 0, 2)).astype(np.float16)
    b1row = np.ascontiguousarray((b1 * gs[:, 0] * S)[None, :]).astype(np.float16)
    sel8 = np.zeros((BS, 16), np.float16)
    sel8[np.arange(BS), np.arange(BS)] = 1.0
    one8 = np.zeros((1, 16), np.float16)
    one8[0, :BS] = 1.0

    phiT = (f("phi_w") * scale * 0.5).astype(np.float16)  # [d_out, h_in]
    psiT = f("psi_w").T
    psi_b = f("psi_b")
    psibT = np.ascontiguousarray(psi_b.reshape(KT_H, 128).T)
    psib_bc = np.ascontiguousarray(np.tile(psi_b[None, :], (128, 1)))
    phibT = np.ascontiguousarray(
        ((f("phi_b") * scale).reshape(KT_H, 128).T).astype(np.float16)
    )
    oW = f("out_w").copy()
    oW[:, :H] *= 0.5  # s1 history stored as 2*s1
    owT = np.ascontiguousarray(oW.T)
    ob_bc = np.ascontiguousarray(np.tile(f("out_b")[None, :], (128, 1)))

    c16 = lambda x: np.ascontiguousarray(x.astype(np.float16))
    shared = dict(
        W0=pack8(W0), W1=pack8(W1), phiT=c16(phiT),
        psiT=c16(psiT), psib_bc=psib_bc, psibT=psibT, phibT=phibT,
        sel8=sel8, one8=one8, b1row=b1row,
        owT=c16(owT), ob_bc=ob_bc,
    )
    in_maps = []
    for ci in range(NCORES):
        sl = slice(ci * BS, (ci + 1) * BS)
        m = dict(shared)
        m["hT"] = c16(h[sl].reshape(BS * R, H).T)
        m["embT"] = np.ascontiguousarray(embT[:, sl, :])
        in_maps.append(m)
    return in_maps


def gather_output(per_core_outs, T_steps=T):
    """per-core [T*8, V] f16 -> [B, T, V] f32."""
    shards = []
    for o in per_core_outs:
        o = np.asarray(o, np.float32).reshape(T_steps, BS, V)
        shards.append(np.ascontiguousarray(o.transpose(1, 0, 2)))
    return np.concatenate(shards, axis=0)


def kernel(**inputs):
    nc = build_program(T)
    in_maps = host_prep(inputs, T)
    res = run_bass_kernel_spmd(nc, in_maps, list(range(NCORES)))
    return gather_output([res.results[ci]["out"] for ci in range(NCORES)])
